# revision 1
# baseline (speedup 1.0000x reference)
"""Trainium2 Bass kernel for nn_CriticUAVob (attention-pool critic).

Math: for each batch item b (4096 total), two attention-pool branches over
s_b [N=128, 3], then a small MLP.  Key identity used: with P = softmax(S)
row-wise and V = s'Wv',

    mean_n (P V)[n] = (1/N) * c^T s' Wv',   c[m] = sum_n U[n,m] / Z[n]

so V is never materialized.  Per item we compute S^T = s' A~ s'^T (K=4
matmuls), U^T = exp(S^T) on ScalarE, G = U^T^T-weighted sums of s' (one
K=128 matmul whose ones-column yields Z), r = 1/Z, t = G^T r (tiny matmul),
and finally a batched MLP over all items at the end.

Sharding: pure data parallel, batch split across 8 NeuronCores.
"""
import os
import sys
import numpy as np

sys.path.insert(0, "/opt/trn_rl_repo")

import concourse.bass as bass
import concourse.tile as tile
from concourse import bacc, mybir
from concourse import bass_utils
from concourse.masks import make_identity

N_CORES = 8
B = 4096
N = 128
BC = B // N_CORES          # 512 items per core
QUADS = BC // 4            # 128 groups of 4 items
F32 = mybir.dt.float32
AF = mybir.ActivationFunctionType

_cache = {}


def _build():
    nc = bacc.Bacc(
        "TRN2",
        target_bir_lowering=False,
        debug=False,
        enable_asserts=False,
        num_devices=N_CORES,
    )
    s_t = nc.dram_tensor("s", [BC, N, 3], F32, kind="ExternalInput")
    amat_t = nc.dram_tensor("amat", [4, 8], F32, kind="ExternalInput")
    wcrs_t = nc.dram_tensor("wcrs", [4, 64], F32, kind="ExternalInput")
    wctg_t = nc.dram_tensor("wctg", [4, 64], F32, kind="ExternalInput")
    w1_t = nc.dram_tensor("w1", [64, 128], F32, kind="ExternalInput")
    w2_t = nc.dram_tensor("w2", [128, 128], F32, kind="ExternalInput")
    w3_t = nc.dram_tensor("w3", [128, 1], F32, kind="ExternalInput")
    b1_t = nc.dram_tensor("b1", [128, 1], F32, kind="ExternalInput")
    b2_t = nc.dram_tensor("b2", [128, 1], F32, kind="ExternalInput")
    b3_t = nc.dram_tensor("b3rep", [1, BC], F32, kind="ExternalInput")
    out_t = nc.dram_tensor("out", [BC, 1], F32, kind="ExternalOutput")

    s_ap = s_t.ap()

    with tile.TileContext(nc) as tc:
        with (
            tc.tile_pool(name="singles", bufs=1) as singles,
            tc.tile_pool(name="qsb", bufs=3) as qsb,
            tc.tile_pool(name="pst", bufs=4, space="PSUM") as pst,
            tc.tile_pool(name="psmall", bufs=3, space="PSUM") as psmall,
        ):
            ident = singles.tile([128, 128], F32)
            make_identity(nc, ident[:])
            amat = singles.tile([4, 8], F32)
            nc.sync.dma_start(amat[:], amat_t.ap())
            wcrs = singles.tile([4, 64], F32)
            nc.sync.dma_start(wcrs[:], wcrs_t.ap())
            wctg = singles.tile([4, 64], F32)
            nc.sync.dma_start(wctg[:], wctg_t.ap())
            w1 = singles.tile([64, 128], F32)
            nc.sync.dma_start(w1[:], w1_t.ap())
            w2 = singles.tile([128, 128], F32)
            nc.sync.dma_start(w2[:], w2_t.ap())
            w3 = singles.tile([128, 1], F32)
            nc.sync.dma_start(w3[:], w3_t.ap())
            b1 = singles.tile([128, 1], F32)
            nc.sync.dma_start(b1[:], b1_t.ap())
            b2 = singles.tile([128, 1], F32)
            nc.sync.dma_start(b2[:], b2_t.ap())
            b3r = singles.tile([1, BC], F32)
            nc.sync.dma_start(b3r[:], b3_t.ap())
            # T^T accumulator: rows k=0..3, cols = item*2 + branch
            tbig = singles.tile([4, 2 * BC], F32)

            for q in range(QUADS):
                # ---- load 4 items' s as [n, (item, k)] with a ones column
                s_nat = qsb.tile([128, 16], F32, tag="s_nat")
                src = s_ap[q * 4:(q + 1) * 4].rearrange("i n k -> n i k")
                dst = s_nat[:].rearrange("n (i f) -> n i f", i=4)
                nc.sync.dma_start(dst[:, :, 0:3], src)
                nc.gpsimd.memset(dst[:, :, 3:4], 1.0)

                # ---- transpose each item: sT[k, n] (4 rows incl ones row)
                ps_t = psmall.tile([4, 512], F32, tag="ps_sm")
                for i in range(4):
                    nc.tensor.transpose(
                        ps_t[:, i * 128:(i + 1) * 128],
                        s_nat[:, i * 4:(i + 1) * 4],
                        ident[:],
                    )
                sT = qsb.tile([4, 512], F32, tag="sT")
                nc.vector.tensor_copy(sT[:], ps_t[:])

                # ---- BT = A~ s'^T per branch (PSUM reads must be 32-aligned,
                # so two [4,512] tiles rather than one [8,512])
                ps_btr = psmall.tile([4, 512], F32, tag="ps_sm")
                ps_btt = psmall.tile([4, 512], F32, tag="ps_sm")
                nc.tensor.matmul(ps_btr[:], amat[:, 0:4], sT[:])
                nc.tensor.matmul(ps_btt[:], amat[:, 4:8], sT[:])
                bt_rs = qsb.tile([4, 512], F32, tag="bt_rs")
                bt_tg = qsb.tile([4, 512], F32, tag="bt_tg")
                nc.vector.tensor_copy(bt_rs[:], ps_btr[:])
                nc.vector.tensor_copy(bt_tg[:], ps_btt[:])

                # ---- S^T per item per branch, then exp
                st_rs = pst.tile([128, 512], F32, tag="st")
                st_tg = pst.tile([128, 512], F32, tag="st")
                for i in range(4):
                    sl = slice(i * 128, (i + 1) * 128)
                    nc.tensor.matmul(st_rs[:, sl], sT[:, sl], bt_rs[:, sl])
                    nc.tensor.matmul(st_tg[:, sl], sT[:, sl], bt_tg[:, sl])
                ut_rs = qsb.tile([128, 512], F32, tag="ut_rs")
                ut_tg = qsb.tile([128, 512], F32, tag="ut_tg")
                nc.scalar.activation(ut_rs[:], st_rs[:], AF.Exp)
                nc.scalar.activation(ut_tg[:], st_tg[:], AF.Exp)

                # ---- G = sum_m U^T[m,n] * s'[m,k]  -> [n, 4]; col 3 = Z
                ps_g = psmall.tile([128, 32], F32, tag="ps_sm")
                for i in range(4):
                    sl = slice(i * 128, (i + 1) * 128)
                    nsl = slice(i * 4, (i + 1) * 4)
                    c0 = (i * 2) * 4
                    c1 = (i * 2 + 1) * 4
                    nc.tensor.matmul(ps_g[:, c0:c0 + 4], ut_rs[:, sl], s_nat[:, nsl])
                    nc.tensor.matmul(ps_g[:, c1:c1 + 4], ut_tg[:, sl], s_nat[:, nsl])
                g_sb = qsb.tile([128, 32], F32, tag="g_sb")
                nc.vector.tensor_copy(g_sb[:], ps_g[:])
                r_sb = qsb.tile([128, 8], F32, tag="r_sb")
                g3 = g_sb[:].rearrange("n (c f) -> n c f", f=4)
                nc.vector.reciprocal(r_sb[:], g3[:, :, 3])

                # ---- t = G^T r  -> [4, 1] per (item, branch)
                ps_tt = psmall.tile([4, 8], F32, tag="ps_sm")
                for c in range(8):
                    nc.tensor.matmul(
                        ps_tt[:, c:c + 1],
                        g_sb[:, c * 4:(c + 1) * 4],
                        r_sb[:, c:c + 1],
                    )
                nc.vector.tensor_copy(tbig[:, q * 8:(q + 1) * 8], ps_tt[:])

            # ---- batched MLP over all BC items
            tb3 = tbig[:].rearrange("p (b j) -> p j b", j=2)
            ps_h = pst.tile([64, BC], F32, tag="st")
            nc.tensor.matmul(ps_h[:], wcrs[:], tb3[:, 0, :], start=True, stop=False)
            nc.tensor.matmul(ps_h[:], wctg[:], tb3[:, 1, :], start=False, stop=True)
            h_sb = singles.tile([64, BC], F32)
            nc.vector.tensor_copy(h_sb[:], ps_h[:])

            ps_z1 = pst.tile([128, BC], F32, tag="st")
            nc.tensor.matmul(ps_z1[:], w1[:], h_sb[:])
            h1 = singles.tile([128, BC], F32)
            nc.scalar.activation(h1[:], ps_z1[:], AF.Tanh, bias=b1[:])

            ps_z2 = pst.tile([128, BC], F32, tag="st")
            nc.tensor.matmul(ps_z2[:], w2[:], h1[:])
            h2 = singles.tile([128, BC], F32)
            nc.scalar.activation(h2[:], ps_z2[:], AF.Tanh, bias=b2[:])

            ps_z3 = psmall.tile([1, BC], F32, tag="ps_sm")
            nc.tensor.matmul(ps_z3[:], w3[:], h2[:])
            y_sb = singles.tile([1, BC], F32)
            nc.vector.tensor_add(y_sb[:], ps_z3[:], b3r[:])

            nc.sync.dma_start(out_t.ap().rearrange("b o -> o b"), y_sb[:])

    nc.compile()
    return nc


def _host_prep(inputs):
    f = lambda x: np.asarray(x, dtype=np.float32)
    s_obs = f(inputs["s_obs"])

    def aug(W, b):
        return np.vstack([f(W), f(b).reshape(1, -1)])  # [4, dout]

    Wq_rs = aug(inputs["Wq_rs"], inputs["bq_rs"])
    Wk_rs = aug(inputs["Wk_rs"], inputs["bk_rs"])
    Wv_rs = aug(inputs["Wv_rs"], inputs["bv_rs"])
    Wq_tg = aug(inputs["Wq_tg"], inputs["bq_tg"])
    Wk_tg = aug(inputs["Wk_tg"], inputs["bk_tg"])
    Wv_tg = aug(inputs["Wv_tg"], inputs["bv_tg"])

    scale = 1.0 / np.sqrt(16.0)
    # S^T orientation needs A~ = A^T where A = Wq' Wk'^T * scale
    At_rs = (Wq_rs @ Wk_rs.T * scale).T.astype(np.float32)
    At_tg = (Wq_tg @ Wk_tg.T * scale).T.astype(np.float32)
    amat = np.concatenate([At_rs.T, At_tg.T], axis=1).astype(np.float32)  # [4,8]

    wcrs = np.zeros((4, 64), np.float32)
    wctg = np.zeros((4, 64), np.float32)
    wcrs[:, 0:32] = Wv_rs / N
    wctg[:, 32:64] = Wv_tg / N

    w1 = f(inputs["W1"])                       # [64, 128]
    b1 = f(inputs["b1"]).reshape(128, 1)
    w2 = f(inputs["W2"])                       # [128, 128]
    b2 = f(inputs["b2"]).reshape(128, 1)
    w3 = f(inputs["W3"])                       # [128, 1]
    b3rep = np.full((1, BC), float(np.asarray(inputs["b3"]).reshape(-1)[0]),
                    np.float32)

    common = dict(amat=amat, wcrs=wcrs, wctg=wctg, w1=w1, w2=w2, w3=w3,
                  b1=b1, b2=b2, b3rep=b3rep)
    in_maps = []
    for c in range(N_CORES):
        m = dict(common)
        m["s"] = np.ascontiguousarray(s_obs[c * BC:(c + 1) * BC])
        in_maps.append(m)
    return in_maps


def kernel(**inputs):
    if "nc" not in _cache:
        _cache["nc"] = _build()
    nc = _cache["nc"]
    in_maps = _host_prep(inputs)
    trace = os.environ.get("KERNEL_TRACE", "0") == "1"
    res = bass_utils.run_bass_kernel_spmd(
        nc, in_maps, core_ids=list(range(N_CORES)), trace=trace
    )
    _cache["last"] = res
    out = np.concatenate([r["out"] for r in res.results], axis=0)
    return out.astype(np.float32)



# revision 2
# speedup vs baseline: 1.1723x; 1.1723x over previous
"""Trainium2 Bass kernel for nn_CriticUAVob (attention-pool critic).

Math per batch item b: two attention-pool branches over s_b [N=128, 3],
then a tiny MLP.  With X = [x, 1] ([128, 4] augmented), A_b = Wq' Wk'^T/4,
U = exp(X A_b X^T), Z[n] = sum_m U[n, m]:

    pooled_b = (1/N) sum_n softmax-row(n) @ V = (Wv'^T t)/N,
    t[k] = sum_n (1/Z[n]) sum_m U[n, m] X[m, k]

Device layout (orientation: U^T [m partitions, (item, branch, n) free]):
  st  = sTq^T @ btY_bd     one [16,128] x [16,1024] block-diag matmul pair
  ut  = exp(st)            ScalarE, PSUM -> SBUF bf16
  g   = xon_i^T @ ut_i     per item: [Z; Gx; Gy; Gz] rows ([4, 256])
  r   = 1/Z row broadcast to 4 partitions (stream_shuffle)
  t   = reduce_n(g * r)    DVE, -> [4, 8] per quad
All transposes, X@A products, and the Wv/W1 fold are precomputed on host.

Sharding: pure data parallel, batch split across 8 NeuronCores.
"""
import os
import sys
import numpy as np

sys.path.insert(0, "/opt/trn_rl_repo")

import concourse.bass as bass
import concourse.tile as tile
from concourse import bacc, mybir
from concourse import bass_utils
import ml_dtypes

N_CORES = 8
B = 4096
N = 128
BC = B // N_CORES          # 512 items per core
QUADS = BC // 4            # 128 groups of 4 items
F32 = mybir.dt.float32
BF16 = mybir.dt.bfloat16
AF = mybir.ActivationFunctionType
AX = mybir.AxisListType
OP = mybir.AluOpType

_cache = {}


def _build():
    nc = bacc.Bacc(
        "TRN2",
        target_bir_lowering=False,
        debug=False,
        enable_asserts=False,
        num_devices=N_CORES,
    )
    sT_t = nc.dram_tensor("sT", [16, 128 * QUADS], BF16, kind="ExternalInput")
    xon_t = nc.dram_tensor("xon", [128, 16 * QUADS], BF16, kind="ExternalInput")
    bty_t = nc.dram_tensor("bty", [QUADS, 16, 1024], BF16, kind="ExternalInput")
    crs_t = nc.dram_tensor("crs", [4, 128], BF16, kind="ExternalInput")
    ctg_t = nc.dram_tensor("ctg", [4, 128], BF16, kind="ExternalInput")
    w2_t = nc.dram_tensor("w2", [128, 128], BF16, kind="ExternalInput")
    w3_t = nc.dram_tensor("w3", [128, 1], BF16, kind="ExternalInput")
    b1_t = nc.dram_tensor("b1", [128, 1], F32, kind="ExternalInput")
    b2_t = nc.dram_tensor("b2", [128, 1], F32, kind="ExternalInput")
    b3_t = nc.dram_tensor("b3rep", [1, BC], F32, kind="ExternalInput")
    out_t = nc.dram_tensor("out", [BC, 1], F32, kind="ExternalOutput")

    with tile.TileContext(nc) as tc:
        with (
            tc.tile_pool(name="singles", bufs=1) as singles,
            tc.tile_pool(name="btyp", bufs=3) as btyp,
            tc.tile_pool(name="utp", bufs=3) as utp,
            tc.tile_pool(name="smallp", bufs=3) as smallp,
            tc.tile_pool(name="pst", bufs=2, space="PSUM") as pst,
            tc.tile_pool(name="psg", bufs=2, space="PSUM") as psg,
        ):
            sT = singles.tile([16, 128 * QUADS], BF16)
            nc.sync.dma_start(sT[:], sT_t.ap())
            xon = singles.tile([128, 16 * QUADS], BF16)
            nc.sync.dma_start(xon[:], xon_t.ap())
            crs = singles.tile([4, 128], BF16)
            nc.sync.dma_start(crs[:], crs_t.ap())
            ctg = singles.tile([4, 128], BF16)
            nc.sync.dma_start(ctg[:], ctg_t.ap())
            w2 = singles.tile([128, 128], BF16)
            nc.sync.dma_start(w2[:], w2_t.ap())
            w3 = singles.tile([128, 1], BF16)
            nc.sync.dma_start(w3[:], w3_t.ap())
            b1 = singles.tile([128, 1], F32)
            nc.sync.dma_start(b1[:], b1_t.ap())
            b2 = singles.tile([128, 1], F32)
            nc.sync.dma_start(b2[:], b2_t.ap())
            b3r = singles.tile([1, BC], F32)
            nc.sync.dma_start(b3r[:], b3_t.ap())
            # t accumulator: rows k = [junk, x, y, z], cols = (quad, item, br)
            tbig = singles.tile([4, 8 * QUADS], F32)
            # 1/Z staging row (rows 1:4 never read before written via shuffle)
            rsrc = singles.tile([4, 1024], F32)
            nc.gpsimd.memset(rsrc[:], 0.0)

            bty_ap = bty_t.ap()
            shuf0 = [0] * 32

            for q in range(QUADS):
                bty = btyp.tile([16, 1024], BF16, tag="bty")
                nc.sync.dma_start(bty[:], bty_ap[q])

                lhs = sT[:, q * 128:(q + 1) * 128]
                ps0 = pst.tile([128, 512], F32, tag="st")
                ps1 = pst.tile([128, 512], F32, tag="st")
                nc.tensor.matmul(ps0[:], lhs, bty[:, 0:512])
                nc.tensor.matmul(ps1[:], lhs, bty[:, 512:1024])

                ut0 = utp.tile([128, 512], BF16, tag="ut")
                ut1 = utp.tile([128, 512], BF16, tag="ut")
                nc.scalar.activation(ut0[:], ps0[:], AF.Exp)
                nc.scalar.activation(ut1[:], ps1[:], AF.Exp)

                # g: per item [Z; Gx; Gy; Gz] = xon_i^T @ ut_i  -> [4, 256]
                ps_g = psg.tile([4, 1024], F32, tag="g")
                xq = q * 16
                nc.tensor.matmul(ps_g[:, 0:256], xon[:, xq:xq + 4], ut0[:, 0:256])
                nc.tensor.matmul(ps_g[:, 256:512], xon[:, xq + 4:xq + 8],
                                 ut0[:, 256:512])
                nc.tensor.matmul(ps_g[:, 512:768], xon[:, xq + 8:xq + 12],
                                 ut1[:, 0:256])
                nc.tensor.matmul(ps_g[:, 768:1024], xon[:, xq + 12:xq + 16],
                                 ut1[:, 256:512])

                # r4 = broadcast(1/Z) ; t = sum_n (g * r4)
                nc.vector.reciprocal(rsrc[0:1, :], ps_g[0:1, :])
                r4 = smallp.tile([4, 1024], F32, tag="r4")
                nc.vector.stream_shuffle(r4[:], rsrc[:], shuf0)
                gw = smallp.tile([4, 1024], F32, tag="gw")
                nc.vector.tensor_mul(gw[:], ps_g[:], r4[:])
                gw3 = gw[:].rearrange("p (g n) -> p g n", n=128)
                nc.vector.tensor_reduce(
                    tbig[:, q * 8:(q + 1) * 8], gw3, axis=AX.X, op=OP.add
                )

            # ---- batched MLP over all BC items
            tb16 = singles.tile([4, 8 * QUADS], BF16)
            nc.vector.tensor_copy(tb16[:], tbig[:])
            tb3 = tb16[:].rearrange("p (b j) -> p j b", j=2)

            ps_h = pst.tile([128, BC], F32, tag="st")
            nc.tensor.matmul(ps_h[:], crs[:], tb3[:, 0, :], start=True, stop=False)
            nc.tensor.matmul(ps_h[:], ctg[:], tb3[:, 1, :], start=False, stop=True)
            h1 = singles.tile([128, BC], BF16)
            nc.scalar.activation(h1[:], ps_h[:], AF.Tanh, bias=b1[:])

            ps_z2 = pst.tile([128, BC], F32, tag="st")
            nc.tensor.matmul(ps_z2[:], w2[:], h1[:])
            h2 = singles.tile([128, BC], BF16)
            nc.scalar.activation(h2[:], ps_z2[:], AF.Tanh, bias=b2[:])

            ps_z3 = psg.tile([1, BC], F32, tag="g")
            nc.tensor.matmul(ps_z3[:], w3[:], h2[:])
            y_sb = singles.tile([1, BC], F32)
            nc.vector.tensor_add(y_sb[:], ps_z3[:], b3r[:])

            nc.sync.dma_start(out_t.ap().rearrange("b o -> o b"), y_sb[:])

    nc.compile()
    return nc


def _host_prep(inputs):
    f = lambda x: np.asarray(x, dtype=np.float32)
    bf = lambda x: np.ascontiguousarray(x).astype(ml_dtypes.bfloat16)
    s_obs = f(inputs["s_obs"])

    def aug(Wk, bk):
        return np.vstack([f(inputs[Wk]), f(inputs[bk]).reshape(1, -1)])

    Wq_rs, Wk_rs = aug("Wq_rs", "bq_rs"), aug("Wk_rs", "bk_rs")
    Wq_tg, Wk_tg = aug("Wq_tg", "bq_tg"), aug("Wk_tg", "bk_tg")
    Wv_rs, Wv_tg = aug("Wv_rs", "bv_rs"), aug("Wv_tg", "bv_tg")

    scale = 1.0 / np.sqrt(16.0)
    A_rs = (Wq_rs @ Wk_rs.T * scale).astype(np.float32)   # [4, 4]
    A_tg = (Wq_tg @ Wk_tg.T * scale).astype(np.float32)

    W1 = f(inputs["W1"])                                   # [64, 128]
    # Fold Wv (x,y,z rows) / N into W1 halves; bias rows of Wv into b1.
    C_rs = np.zeros((4, 128), np.float32)
    C_tg = np.zeros((4, 128), np.float32)
    C_rs[1:4] = (Wv_rs[0:3] / N) @ W1[0:32]
    C_tg[1:4] = (Wv_tg[0:3] / N) @ W1[32:64]
    b1_eff = (f(inputs["b1"]) + Wv_rs[3] @ W1[0:32] + Wv_tg[3] @ W1[32:64])

    common = dict(
        crs=bf(C_rs), ctg=bf(C_tg),
        w2=bf(f(inputs["W2"])), w3=bf(f(inputs["W3"])),
        b1=b1_eff.reshape(128, 1).astype(np.float32),
        b2=f(inputs["b2"]).reshape(128, 1),
        b3rep=np.full((1, BC), float(np.asarray(inputs["b3"]).reshape(-1)[0]),
                      np.float32),
    )

    in_maps = []
    for c in range(N_CORES):
        s_c = s_obs[c * BC:(c + 1) * BC]                   # [512, 128, 3]
        Xa = np.concatenate([s_c, np.ones((BC, N, 1), np.float32)], axis=2)

        # sT_all [16, QUADS*128]: rows (i, j) j-order [x, y, z, 1]
        sT = Xa.reshape(QUADS, 4, N, 4).transpose(1, 3, 0, 2).reshape(16, -1)

        # xon_all [128, QUADS*16]: cols (q, i, k) k-order [1, x, y, z]
        Xon = np.concatenate([np.ones((BC, N, 1), np.float32), s_c], axis=2)
        xon = Xon.transpose(1, 0, 2).reshape(N, -1)

        # btY block-diag [QUADS, 16, 1024]: diag block i = Y^T [4, (br, n)]
        Y = np.stack([Xa @ A_rs, Xa @ A_tg], axis=1)       # [512, 2, 128, 4]
        blocks = Y.transpose(0, 3, 1, 2).reshape(BC, 4, 256)
        bd = np.zeros((QUADS, 16, 1024), np.float32)
        bdv = bd.reshape(QUADS, 4, 4, 4, 256)
        blv = blocks.reshape(QUADS, 4, 4, 256)
        for i in range(4):
            bdv[:, i, :, i, :] = blv[:, i]

        m = dict(common)
        m["sT"] = bf(sT)
        m["xon"] = bf(xon)
        m["bty"] = bf(bd)
        in_maps.append(m)
    return in_maps


def kernel(**inputs):
    if "nc" not in _cache:
        _cache["nc"] = _build()
    nc = _cache["nc"]
    in_maps = _host_prep(inputs)
    trace = os.environ.get("KERNEL_TRACE", "0") == "1"
    res = bass_utils.run_bass_kernel_spmd(
        nc, in_maps, core_ids=list(range(N_CORES)), trace=trace
    )
    _cache["last"] = res
    out = np.concatenate([r["out"] for r in res.results], axis=0)
    return out.astype(np.float32)


# revision 8
# speedup vs baseline: 7.4203x; 6.3296x over previous
"""Trainium2 Bass kernel for nn_CriticUAVob (attention-pool critic).

Math per batch item: two attention-pool branches over s [N=128, 3], then a
tiny MLP.  With X = [x, 1] ([128, 4] augmented), A_b = Wq' Wk'^T/4,
U = exp(X A_b X^T), Z[n] = sum_m U[n, m]:

    pooled_b = (Wv'^T t)/N,   t[k] = sum_n (1/Z[n]) sum_m U[n, m] X[m, k]

Device dataflow per quad of 4 items (U^T layout: m on partitions):
  st [128(m), (i,b,n)] = sTq^T @ btY_bd   (host-prepped block-diag rhs)
  ut = exp(st)                            ScalarE, PSUM -> SBUF bf16
  G [128(n), 4] per (i,b) = ut_blk^T @ xon_i   (k = [1,x,y,z], Z at k=0)
  r [128, 8] = 1/Z ; gw = G * r           wide DVE ops only
  t [128, 1] = gw^T @ ones                one matmul; partitions 32i+4b+k
MLP stage 1 contracts (b, k) per i-strip (32-aligned), so quads are formed
as items {q, 128+q, 256+q, 384+q} to make output columns land in order.
All transposes / X@A products / Wv-W1 folds precomputed on host; b3 is
added on host after gather.

Sharding: pure data parallel, batch split across 8 NeuronCores.
"""
import os
import sys
import numpy as np

sys.path.insert(0, "/opt/trn_rl_repo")

import concourse.bass as bass
import concourse.tile as tile
from concourse import bacc, mybir
from concourse import bass_utils
import ml_dtypes

N_CORES = 8
B = 4096
N = 128
BC = B // N_CORES          # 512 items per core
QUADS = BC // 4            # 128 groups of 4 items
F32 = mybir.dt.float32
BF16 = mybir.dt.bfloat16
AF = mybir.ActivationFunctionType

_cache = {}


def _build():
    nc = bacc.Bacc(
        "TRN2",
        target_bir_lowering=False,
        debug=False,
        enable_asserts=False,
        num_devices=N_CORES,
    )
    sT_t = nc.dram_tensor("sT", [16, 128 * QUADS], BF16, kind="ExternalInput")
    xon_t = nc.dram_tensor("xon", [128, 16 * QUADS], BF16, kind="ExternalInput")
    bty_t = nc.dram_tensor("bty", [QUADS, 16, 1024], BF16, kind="ExternalInput")
    cc_t = nc.dram_tensor("cc", [128, 512], BF16, kind="ExternalInput")
    w2_t = nc.dram_tensor("w2", [128, 128], BF16, kind="ExternalInput")
    w3_t = nc.dram_tensor("w3", [128, 1], BF16, kind="ExternalInput")
    b1_t = nc.dram_tensor("b1", [128, 1], F32, kind="ExternalInput")
    b2_t = nc.dram_tensor("b2", [128, 1], F32, kind="ExternalInput")
    out_t = nc.dram_tensor("out", [BC, 1], F32, kind="ExternalOutput")

    with tile.TileContext(nc) as tc:
        with (
            tc.tile_pool(name="singles", bufs=1) as singles,
            tc.tile_pool(name="btyp", bufs=3) as btyp,
            tc.tile_pool(name="utp", bufs=4) as utp,
            tc.tile_pool(name="smallp", bufs=3) as smallp,
            tc.tile_pool(name="pst", bufs=4, space="PSUM") as pst,
            tc.tile_pool(name="psg", bufs=2, space="PSUM") as psg,
            tc.tile_pool(name="ptt", bufs=2, space="PSUM") as ptt,
        ):
            sT = singles.tile([16, 128 * QUADS], BF16)
            nc.sync.dma_start(sT[:], sT_t.ap())
            xon = singles.tile([128, 16 * QUADS], BF16)
            nc.sync.dma_start(xon[:], xon_t.ap())
            cc = singles.tile([128, 512], BF16)
            nc.sync.dma_start(cc[:], cc_t.ap())
            w2 = singles.tile([128, 128], BF16)
            nc.sync.dma_start(w2[:], w2_t.ap())
            w3 = singles.tile([128, 1], BF16)
            nc.sync.dma_start(w3[:], w3_t.ap())
            b1 = singles.tile([128, 1], F32)
            nc.sync.dma_start(b1[:], b1_t.ap())
            b2 = singles.tile([128, 1], F32)
            nc.sync.dma_start(b2[:], b2_t.ap())

            ones = singles.tile([128, 1], BF16)
            nc.gpsimd.memset(ones[:], 1.0)
            # t accumulator: rows 32i+4b+k, cols = quad
            tbig = singles.tile([128, QUADS], BF16)
            # gw double buffer: only cols {32i+4b+k} ever written; rest stay 0
            gw_a = singles.tile([128, 128], BF16)
            gw_b = singles.tile([128, 128], BF16)
            gws = [gw_a, gw_b]
            nc.gpsimd.memset(gw_a[:], 0.0)
            nc.gpsimd.memset(gw_b[:], 0.0)

            bty_ap = bty_t.ap()

            for q in range(QUADS):
                bty = btyp.tile([16, 1024], BF16, tag="bty")
                nc.sync.dma_start(bty[:], bty_ap[q])

                lhs = sT[:, q * 128:(q + 1) * 128]
                ps0 = pst.tile([128, 512], F32, tag="st")
                ps1 = pst.tile([128, 512], F32, tag="st")
                nc.tensor.matmul(ps0[:], lhs, bty[:, 0:512])
                nc.tensor.matmul(ps1[:], lhs, bty[:, 512:1024])

                ut0 = utp.tile([128, 512], BF16, tag="ut")
                ut1 = utp.tile([128, 512], BF16, tag="ut")
                nc.scalar.activation(ut0[:], ps0[:], AF.Exp)
                nc.scalar.activation(ut1[:], ps1[:], AF.Exp)

                # G per (i,b): [128(n), 4(k)] at ps_g col 32i+4b
                ps_g = psg.tile([128, 128], F32, tag="g")
                xq = q * 16
                for i in range(4):
                    xsl = xon[:, xq + i * 4:xq + (i + 1) * 4]
                    ut = ut0 if i < 2 else ut1
                    c0 = (i % 2) * 256
                    nc.tensor.matmul(
                        ps_g[:, 32 * i:32 * i + 4], ut[:, c0:c0 + 128], xsl
                    )
                    nc.tensor.matmul(
                        ps_g[:, 32 * i + 4:32 * i + 8],
                        ut[:, c0 + 128:c0 + 256], xsl,
                    )

                # r = 1/Z (Z at k=0 of each (i,b) block), gw = G * r
                g4 = ps_g[:].rearrange("p (i b k) -> p i b k", i=4, b=8)
                r = smallp.tile([128, 8], F32, tag="r")
                r2 = r[:].rearrange("p (i b) -> p i b", i=4)
                nc.vector.reciprocal(r2, g4[:, :, 0:2, 0])
                gw = gws[q % 2]
                for c in range(8):
                    i, b = c // 2, c % 2
                    o = 32 * i + 4 * b
                    nc.vector.tensor_scalar_mul(
                        gw[:, o:o + 4], ps_g[:, o:o + 4], r[:, c:c + 1]
                    )

                # t = sum_n gw  -> [128, 1] at partitions 32i+4b+k
                ps_t = ptt.tile([128, 1], F32, tag="t")
                nc.tensor.matmul(ps_t[:], gw[:], ones[:])
                nc.vector.tensor_copy(tbig[:, q:q + 1], ps_t[:])

            # ---- batched MLP; item of (q, i) is i*128+q so cols are ordered
            ps_h = pst.tile([128, BC], F32, tag="st")
            for i in range(4):
                nc.tensor.matmul(
                    ps_h[:, i * 128:(i + 1) * 128],
                    cc[:, i * 128:(i + 1) * 128],
                    tbig[:],
                )
            h1 = singles.tile([128, BC], BF16)
            nc.scalar.activation(h1[:], ps_h[:], AF.Tanh, bias=b1[:])

            ps_z2 = pst.tile([128, BC], F32, tag="st")
            nc.tensor.matmul(ps_z2[:], w2[:], h1[:])
            h2 = singles.tile([128, BC], BF16)
            nc.scalar.activation(h2[:], ps_z2[:], AF.Tanh, bias=b2[:])

            ps_z3 = psg.tile([1, BC], F32, tag="g")
            nc.tensor.matmul(ps_z3[:], w3[:], h2[:])
            y_sb = singles.tile([1, BC], F32)
            nc.vector.tensor_copy(y_sb[:], ps_z3[:])

            nc.sync.dma_start(out_t.ap().rearrange("b o -> o b"), y_sb[:])

    nc.compile()
    return nc


def _host_prep(inputs):
    f = lambda x: np.asarray(x, dtype=np.float32)
    bf = lambda x: np.ascontiguousarray(x).astype(ml_dtypes.bfloat16)
    s_obs = f(inputs["s_obs"])

    def aug(Wk, bk):
        return np.vstack([f(inputs[Wk]), f(inputs[bk]).reshape(1, -1)])

    Wq_rs, Wk_rs = aug("Wq_rs", "bq_rs"), aug("Wk_rs", "bk_rs")
    Wq_tg, Wk_tg = aug("Wq_tg", "bq_tg"), aug("Wk_tg", "bk_tg")
    Wv_rs, Wv_tg = aug("Wv_rs", "bv_rs"), aug("Wv_tg", "bv_tg")

    scale = 1.0 / np.sqrt(16.0)
    A_rs = (Wq_rs @ Wk_rs.T * scale).astype(np.float32)   # [4, 4]
    A_tg = (Wq_tg @ Wk_tg.T * scale).astype(np.float32)

    W1 = f(inputs["W1"])                                   # [64, 128]
    # cc rows 4b+k (replicated per 32-strip): C_b[k] = (Wv_b[k-1]/N) @ W1blk
    CC = np.zeros((8, 128), np.float32)
    CC[1:4] = (Wv_rs[0:3] / N) @ W1[0:32]
    CC[5:8] = (Wv_tg[0:3] / N) @ W1[32:64]
    # cc variant i (cols i*128..): nonzero only in partition strip 32i..32i+8
    cc = np.zeros((128, 512), np.float32)
    for i in range(4):
        cc[32 * i:32 * i + 8, i * 128:(i + 1) * 128] = CC
    b1_eff = (f(inputs["b1"]) + Wv_rs[3] @ W1[0:32] + Wv_tg[3] @ W1[32:64])

    common = dict(
        cc=bf(cc),
        w2=bf(f(inputs["W2"])), w3=bf(f(inputs["W3"])),
        b1=b1_eff.reshape(128, 1).astype(np.float32),
        b2=f(inputs["b2"]).reshape(128, 1),
    )

    # quad q = items {q, 128+q, 256+q, 384+q}: position (q, i) -> i*128+q
    order = np.arange(BC).reshape(4, QUADS).T.reshape(-1)

    in_maps = []
    for c in range(N_CORES):
        s_c = s_obs[c * BC:(c + 1) * BC][order]            # [512, 128, 3]
        Xa = np.concatenate([s_c, np.ones((BC, N, 1), np.float32)], axis=2)

        # sT_all [16, QUADS*128]: rows (i, j) j-order [x, y, z, 1]
        sT = Xa.reshape(QUADS, 4, N, 4).transpose(1, 3, 0, 2).reshape(16, -1)

        # xon_all [128, QUADS*16]: cols (q, i, k) k-order [1, x, y, z]
        Xon = np.concatenate([np.ones((BC, N, 1), np.float32), s_c], axis=2)
        xon = Xon.transpose(1, 0, 2).reshape(N, -1)

        # btY block-diag [QUADS, 16, 1024]: diag block i = Y^T [4, (br, n)]
        Y = np.stack([Xa @ A_rs, Xa @ A_tg], axis=1)       # [512, 2, 128, 4]
        blocks = Y.transpose(0, 3, 1, 2).reshape(BC, 4, 256)
        bd = np.zeros((QUADS, 16, 1024), np.float32)
        bdv = bd.reshape(QUADS, 4, 4, 4, 256)
        blv = blocks.reshape(QUADS, 4, 4, 256)
        for i in range(4):
            bdv[:, i, :, i, :] = blv[:, i]

        m = dict(common)
        m["sT"] = bf(sT)
        m["xon"] = bf(xon)
        m["bty"] = bf(bd)
        in_maps.append(m)
    return in_maps


def kernel(**inputs):
    if "nc" not in _cache:
        _cache["nc"] = _build()
    nc = _cache["nc"]
    in_maps = _host_prep(inputs)
    trace = os.environ.get("KERNEL_TRACE", "0") == "1"
    res = bass_utils.run_bass_kernel_spmd(
        nc, in_maps, core_ids=list(range(N_CORES)), trace=trace
    )
    _cache["last"] = res
    b3 = float(np.asarray(inputs["b3"]).reshape(-1)[0])
    out = np.concatenate([r["out"] for r in res.results], axis=0) + b3
    return out.astype(np.float32)


# revision 10
# speedup vs baseline: 9.0952x; 1.2257x over previous
"""Trainium2 Bass kernel for nn_CriticUAVob (attention-pool critic).

Math per batch item: two attention-pool branches over s [N=128, 3], then a
tiny MLP.  With X = [x, 1] ([128, 4] augmented), A_b = Wq' Wk'^T/4,
U = exp(X A_b X^T), Z[n] = sum_m U[n, m]:

    pooled_b = (Wv'^T t)/N,   t[k] = sum_n (1/Z[n]) sum_m U[n, m] X[m, k]

Device dataflow per quad of 4 items (U^T layout: m on partitions):
  st [128(m), (i,b,n)] = sTq^T @ btY_bd   (host-prepped block-diag rhs)
  ut = exp(st)                            ScalarE, PSUM -> SBUF bf16
  G [128(n), 4] per (i,b) = ut_blk^T @ xon_i   (k = [1,x,y,z], Z at k=0)
  r [128, 8] = 1/Z ; gw = G * r           wide DVE ops only
  t [128, 1] = gw^T @ ones                one matmul; partitions 32i+4b+k
MLP stage 1 contracts (b, k) per i-strip (32-aligned), so quads are formed
as items {q, 128+q, 256+q, 384+q} to make output columns land in order.
All transposes / X@A products / Wv-W1 folds precomputed on host; b3 is
added on host after gather.

Sharding: pure data parallel, batch split across 8 NeuronCores.
"""
import os
import sys
import numpy as np

sys.path.insert(0, "/opt/trn_rl_repo")

import concourse.bass as bass
import concourse.tile as tile
from concourse import bacc, mybir
from concourse import bass_utils
import ml_dtypes

N_CORES = 8
B = 4096
N = 128
BC = B // N_CORES          # 512 items per core
QUADS = BC // 4            # 128 groups of 4 items
F32 = mybir.dt.float32
BF16 = mybir.dt.bfloat16
AF = mybir.ActivationFunctionType

_cache = {}


def _build():
    nc = bacc.Bacc(
        "TRN2",
        target_bir_lowering=False,
        debug=False,
        enable_asserts=False,
        num_devices=N_CORES,
    )
    sT_t = nc.dram_tensor("sT", [16, 128 * QUADS], BF16, kind="ExternalInput")
    xon_t = nc.dram_tensor("xon", [128, 16 * QUADS], BF16, kind="ExternalInput")
    bty_t = nc.dram_tensor("bty", [QUADS, 16, 1024], BF16, kind="ExternalInput")
    cc_t = nc.dram_tensor("cc", [128, 512], BF16, kind="ExternalInput")
    w2_t = nc.dram_tensor("w2", [128, 128], BF16, kind="ExternalInput")
    w3_t = nc.dram_tensor("w3", [128, 1], BF16, kind="ExternalInput")
    b1_t = nc.dram_tensor("b1", [128, 1], F32, kind="ExternalInput")
    b2_t = nc.dram_tensor("b2", [128, 1], F32, kind="ExternalInput")
    out_t = nc.dram_tensor("out", [BC, 1], F32, kind="ExternalOutput")

    with tile.TileContext(nc) as tc:
        with (
            tc.tile_pool(name="singles", bufs=1) as singles,
            tc.tile_pool(name="btyp", bufs=3) as btyp,
            tc.tile_pool(name="utp", bufs=4) as utp,
            tc.tile_pool(name="smallp", bufs=3) as smallp,
            tc.tile_pool(name="pst", bufs=2, space="PSUM") as pst,
            tc.tile_pool(name="psg", bufs=2, space="PSUM") as psg,
            tc.tile_pool(name="ptt", bufs=2, space="PSUM") as ptt,
        ):
            sT = singles.tile([16, 128 * QUADS], BF16)
            nc.sync.dma_start(sT[:], sT_t.ap())
            xon = singles.tile([128, 16 * QUADS], BF16)
            nc.sync.dma_start(xon[:], xon_t.ap())
            cc = singles.tile([128, 512], BF16)
            nc.sync.dma_start(cc[:], cc_t.ap())
            w2 = singles.tile([128, 128], BF16)
            nc.sync.dma_start(w2[:], w2_t.ap())
            w3 = singles.tile([128, 1], BF16)
            nc.sync.dma_start(w3[:], w3_t.ap())
            b1 = singles.tile([128, 1], F32)
            nc.sync.dma_start(b1[:], b1_t.ap())
            b2 = singles.tile([128, 1], F32)
            nc.sync.dma_start(b2[:], b2_t.ap())

            ones = singles.tile([128, 1], BF16)
            nc.gpsimd.memset(ones[:], 1.0)
            # t accumulator: rows 32i+4b+k, cols = quad
            tbig = singles.tile([128, QUADS], BF16)
            # gw double buffer: only cols {32i+4b+k} ever written; rest stay 0
            gw_a = singles.tile([128, 128], BF16)
            gw_b = singles.tile([128, 128], BF16)
            gws = [gw_a, gw_b]
            nc.gpsimd.memset(gw_a[:], 0.0)
            nc.gpsimd.memset(gw_b[:], 0.0)

            bty_ap = bty_t.ap()

            for q in range(QUADS):
                bty = btyp.tile([16, 1024], BF16, tag="bty")
                nc.sync.dma_start(bty[:], bty_ap[q])

                lhs = sT[:, q * 128:(q + 1) * 128]
                ps = pst.tile([128, 1024], F32, tag="st")
                nc.tensor.matmul(ps[:, 0:512], lhs, bty[:, 0:512])
                nc.tensor.matmul(ps[:, 512:1024], lhs, bty[:, 512:1024])

                ut = utp.tile([128, 1024], BF16, tag="ut")
                nc.scalar.activation(ut[:], ps[:], AF.Exp)

                # G per (i,b): [128(n), 4(k)] at ps_g col 32i+4b
                ps_g = psg.tile([128, 128], F32, tag="g")
                xq = q * 16
                for i in range(4):
                    xsl = xon[:, xq + i * 4:xq + (i + 1) * 4]
                    c0 = i * 256
                    nc.tensor.matmul(
                        ps_g[:, 32 * i:32 * i + 4], ut[:, c0:c0 + 128], xsl
                    )
                    nc.tensor.matmul(
                        ps_g[:, 32 * i + 4:32 * i + 8],
                        ut[:, c0 + 128:c0 + 256], xsl,
                    )

                # r = 1/Z (Z at k=0 of each (i,b) block), gw = G * r
                g4 = ps_g[:].rearrange("p (i b k) -> p i b k", i=4, b=8)
                r = smallp.tile([128, 8], F32, tag="r")
                r2 = r[:].rearrange("p (i b) -> p i b", i=4)
                nc.vector.reciprocal(r2, g4[:, :, 0:2, 0])
                gw = gws[q % 2]
                gw4 = gw[:].rearrange("p (i b k) -> p i b k", i=4, b=8)
                rb = r2.unsqueeze(3).broadcast_to([128, 4, 2, 4])
                nc.vector.tensor_mul(gw4[:, :, 0:2, :], g4[:, :, 0:2, :], rb)

                # t = sum_n gw  -> [128, 1] at partitions 32i+4b+k
                ps_t = ptt.tile([128, 1], F32, tag="t")
                nc.tensor.matmul(ps_t[:], gw[:], ones[:])
                nc.vector.tensor_copy(tbig[:, q:q + 1], ps_t[:])

            # ---- batched MLP; item of (q, i) is i*128+q so cols are ordered
            ps_h = pst.tile([128, BC], F32, tag="st")
            for i in range(4):
                nc.tensor.matmul(
                    ps_h[:, i * 128:(i + 1) * 128],
                    cc[:, i * 128:(i + 1) * 128],
                    tbig[:],
                )
            h1 = singles.tile([128, BC], BF16)
            nc.scalar.activation(h1[:], ps_h[:], AF.Tanh, bias=b1[:])

            ps_z2 = pst.tile([128, BC], F32, tag="st")
            nc.tensor.matmul(ps_z2[:], w2[:], h1[:])
            h2 = singles.tile([128, BC], BF16)
            nc.scalar.activation(h2[:], ps_z2[:], AF.Tanh, bias=b2[:])

            ps_z3 = psg.tile([1, BC], F32, tag="g")
            nc.tensor.matmul(ps_z3[:], w3[:], h2[:])
            y_sb = singles.tile([1, BC], F32)
            nc.vector.tensor_copy(y_sb[:], ps_z3[:])

            nc.sync.dma_start(out_t.ap().rearrange("b o -> o b"), y_sb[:])

    nc.compile()
    return nc


def _host_prep(inputs):
    f = lambda x: np.asarray(x, dtype=np.float32)
    bf = lambda x: np.ascontiguousarray(x).astype(ml_dtypes.bfloat16)
    s_obs = f(inputs["s_obs"])

    def aug(Wk, bk):
        return np.vstack([f(inputs[Wk]), f(inputs[bk]).reshape(1, -1)])

    Wq_rs, Wk_rs = aug("Wq_rs", "bq_rs"), aug("Wk_rs", "bk_rs")
    Wq_tg, Wk_tg = aug("Wq_tg", "bq_tg"), aug("Wk_tg", "bk_tg")
    Wv_rs, Wv_tg = aug("Wv_rs", "bv_rs"), aug("Wv_tg", "bv_tg")

    scale = 1.0 / np.sqrt(16.0)
    A_rs = (Wq_rs @ Wk_rs.T * scale).astype(np.float32)   # [4, 4]
    A_tg = (Wq_tg @ Wk_tg.T * scale).astype(np.float32)

    W1 = f(inputs["W1"])                                   # [64, 128]
    # cc rows 4b+k (replicated per 32-strip): C_b[k] = (Wv_b[k-1]/N) @ W1blk
    CC = np.zeros((8, 128), np.float32)
    CC[1:4] = (Wv_rs[0:3] / N) @ W1[0:32]
    CC[5:8] = (Wv_tg[0:3] / N) @ W1[32:64]
    # cc variant i (cols i*128..): nonzero only in partition strip 32i..32i+8
    cc = np.zeros((128, 512), np.float32)
    for i in range(4):
        cc[32 * i:32 * i + 8, i * 128:(i + 1) * 128] = CC
    b1_eff = (f(inputs["b1"]) + Wv_rs[3] @ W1[0:32] + Wv_tg[3] @ W1[32:64])

    common = dict(
        cc=bf(cc),
        w2=bf(f(inputs["W2"])), w3=bf(f(inputs["W3"])),
        b1=b1_eff.reshape(128, 1).astype(np.float32),
        b2=f(inputs["b2"]).reshape(128, 1),
    )

    # quad q = items {q, 128+q, 256+q, 384+q}: position (q, i) -> i*128+q
    order = np.arange(BC).reshape(4, QUADS).T.reshape(-1)

    in_maps = []
    for c in range(N_CORES):
        s_c = s_obs[c * BC:(c + 1) * BC][order]            # [512, 128, 3]
        Xa = np.concatenate([s_c, np.ones((BC, N, 1), np.float32)], axis=2)

        # sT_all [16, QUADS*128]: rows (i, j) j-order [x, y, z, 1]
        sT = Xa.reshape(QUADS, 4, N, 4).transpose(1, 3, 0, 2).reshape(16, -1)

        # xon_all [128, QUADS*16]: cols (q, i, k) k-order [1, x, y, z]
        Xon = np.concatenate([np.ones((BC, N, 1), np.float32), s_c], axis=2)
        xon = Xon.transpose(1, 0, 2).reshape(N, -1)

        # btY block-diag [QUADS, 16, 1024]: diag block i = Y^T [4, (br, n)]
        Y = np.stack([Xa @ A_rs, Xa @ A_tg], axis=1)       # [512, 2, 128, 4]
        blocks = Y.transpose(0, 3, 1, 2).reshape(BC, 4, 256)
        bd = np.zeros((QUADS, 16, 1024), np.float32)
        bdv = bd.reshape(QUADS, 4, 4, 4, 256)
        blv = blocks.reshape(QUADS, 4, 4, 256)
        for i in range(4):
            bdv[:, i, :, i, :] = blv[:, i]

        m = dict(common)
        m["sT"] = bf(sT)
        m["xon"] = bf(xon)
        m["bty"] = bf(bd)
        in_maps.append(m)
    return in_maps


def kernel(**inputs):
    if "nc" not in _cache:
        _cache["nc"] = _build()
    nc = _cache["nc"]
    in_maps = _host_prep(inputs)
    trace = os.environ.get("KERNEL_TRACE", "0") == "1"
    res = bass_utils.run_bass_kernel_spmd(
        nc, in_maps, core_ids=list(range(N_CORES)), trace=trace
    )
    _cache["last"] = res
    b3 = float(np.asarray(inputs["b3"]).reshape(-1)[0])
    out = np.concatenate([r["out"] for r in res.results], axis=0) + b3
    return out.astype(np.float32)
